# revision 45
# baseline (speedup 1.0000x reference)
"""Masked dot-product attention on 8 Trainium2 NeuronCores.

Problem shapes (hardcoded): queries/keys/values [128, 1024, 64] f32,
valid_lens [8] int (per-batch key valid length; BH = 8 batches x 16 heads).

Sharding: core c handles heads {b*16 + 2c, b*16 + 2c + 1} for all batches b
(16 heads/core, every batch present on every core -> uniform work, and one
compiled program serves all cores even with valid-len-dependent trip counts).

Host-side input prep (layout only; all attention math runs on device):
  - Q^T [BH, 64, 1024] with columns in "paired" order: column c*128+p holds
    query position (c//2)*256 + 2p + (c%2), so the output DMA writes >=512B
    contiguous runs (2x DMA bandwidth); the permutation is undone by the
    output access pattern. K^T [BH, 64, 1024] in natural column order
    (contiguous rows already give full DMA bandwidth, and natural order
    keeps valid-len truncation at 128-chunk granularity).
  - V is augmented with a ones column (softmax-denominator trick):
    [BH, 8, 128, 65], chunk-major.
  - mask is an additive bias laid out exactly as the device consumes it:
    [128, b*8+c] = 0 or -1e6 for key position c*128+p of batch b.

Per-head device pipeline (scores kept transposed, S^T[k, q]):
  per k-chunk c (only chunks below the batch's valid_len are computed):
    S^T[c] [128, 1024] = K^T_c.T @ Q^T            (PSUM, 2 matmuls, fp32r)
    P^T[c] = exp(S^T[c] * 1/8 + maskbias_c)       (ScalarE, bias = mask col)
  PV with ones-augmented V: out^T [65, q] += [V|1]_c.T @ P^T[c]; row 64
  accumulates sum(exp) = softmax denominator.
  PE-transpose out^T back to [q, 65]; reciprocal + scale on DVE -> [q, 64];
  DMA out (descriptors un-permute q).

fp32r (TF32-like, 4-byte) matmul inputs: 4x faster than fp32 on the PE,
HW-measured end-to-end rel err ~2e-4. No max-subtraction needed: scores are
O(10) so exp never overflows, masked entries give exactly 0. Fully-masked
batches (valid_len == 0) are patched on host to the reference's
uniform-softmax value.
"""

import numpy as np

P = 128          # partitions / k-chunk size
D = 64           # head dim
QL = 1024        # query length
KL = 1024        # key length
NB = 8           # batches
NH = 16          # heads per batch
NCORES = 8
HPC = 16         # heads per core
NCHUNK = KL // P # 8 k-chunks
NEG = -1.0e6

_POOLCFG = dict(io=3, pt=2, ot=2, fin=2, s=2, o=2, f=2)


def _split_excess_waits(nc, max_waits=1):
    """This walrus (gen3) accepts only one sync-wait per instruction, but Tile
    emits up to 2 on compute ops and 5+ on the kernel-tail drain. Hoist excess
    on_wait entries onto fresh InstEventSemaphore ops on the same engine,
    inserted immediately before the offending instruction (same semantics:
    the engine stalls on each wait sequentially)."""
    import bass_rust
    import concourse.mybir as mybir

    n_split = 0
    for func in nc.m.functions:
        for block in func.blocks:
            out = []
            changed = False
            for inst in block.instructions:
                si = getattr(inst, "sync_info", None)
                waits = list(si.on_wait) if si is not None else []
                if len(waits) > max_waits:
                    changed = True
                    for w in waits[:-max_waits]:
                        n_split += 1
                        out.append(
                            mybir.InstEventSemaphore(
                                name=f"waitsplit_{n_split}_{inst.name}",
                                engine=inst.engine,
                                ins=[],
                                outs=[],
                                sync_info=bass_rust.SyncInfo(
                                    on_wait=[w], on_update=[]
                                ),
                            )
                        )
                    inst.sync_info = bass_rust.SyncInfo(
                        on_wait=waits[-max_waits:], on_update=list(si.on_update)
                    )
                out.append(inst)
            if changed:
                block.instructions = out
    return n_split


def _build(nc_chunks=None, reps=1):
    """Build the Bass program. nc_chunks: per-batch count of 128-wide k-chunks
    to compute (valid-len truncation). reps>1 repeats the whole pipeline
    in-NEFF (only used for wall-clock delta timing experiments)."""
    import concourse.bass as bass
    import concourse.mybir as mybir
    from concourse.tile import TileContext
    from concourse.masks import make_identity

    if nc_chunks is None:
        nc_chunks = [NCHUNK] * NB

    f32 = mybir.dt.float32
    f32r = mybir.dt.float32r
    Exp = mybir.ActivationFunctionType.Exp

    nc = bass.Bass(trn_type="TRN2")
    qd = nc.dram_tensor("qt", [HPC, D, QL], f32r, kind="ExternalInput")
    kd = nc.dram_tensor("kt", [HPC, D, KL], f32r, kind="ExternalInput")
    vd = nc.dram_tensor("v", [HPC, NCHUNK, P, D + 1], f32r, kind="ExternalInput")
    md = nc.dram_tensor("mask", [P, NB * NCHUNK], f32, kind="ExternalInput")
    od = nc.dram_tensor("out", [HPC, QL, D], f32, kind="ExternalOutput")

    cfg = dict(_POOLCFG)
    with TileContext(nc) as tc:
        with (
            tc.tile_pool(name="consts", bufs=1) as consts,
            tc.tile_pool(name="io", bufs=cfg["io"]) as io,
            tc.tile_pool(name="pt", bufs=cfg["pt"]) as ptp,
            tc.tile_pool(name="ot", bufs=cfg["ot"]) as otp,
            tc.tile_pool(name="fin", bufs=cfg["fin"]) as finp,
            tc.tile_pool(name="rc", bufs=4) as rcp,
            tc.tile_pool(name="ps_s", bufs=cfg["s"], space="PSUM") as ps_s,
            tc.tile_pool(name="ps_o", bufs=cfg["o"], space="PSUM") as ps_o,
            tc.tile_pool(name="ps_f", bufs=cfg["f"], space="PSUM") as ps_f,
        ):
            identity = consts.tile([P, P], f32)
            make_identity(nc, identity)
            # prime the ScalarE exp table load (~2.7us) so it overlaps the
            # first input DMAs instead of stalling the first real exp
            scratch = consts.tile([1, 1], f32)
            nc.vector.memset(scratch, 0.0)
            nc.scalar.activation(scratch, scratch, Exp)
            # prime the PE clock ramp with short dummy matmuls sized to end
            # right as the first real operands land (PE is FIFO: too many
            # dummies would delay the real matmuls)
            warm = ps_f.tile([1, D], f32, tag="pf")
            for _ in range(7):
                nc.tensor.matmul(
                    warm, identity[:, 0:1], identity[:, 0:D],
                    start=True, stop=True,
                )
            mask_sb = consts.tile([P, NB, NCHUNK], f32)

            def emit_mask():
                # SWDGE via the idle GPSIMD engine: issues in parallel with
                # the SP-sequencer DMAs feeding the first matmuls
                nc.gpsimd.dma_start(
                    out=mask_sb, in_=md.rearrange("p (b c) -> p b c", b=NB)
                )

            def emit_front(h, first=False):
                b = h // 2
                nck = nc_chunks[b]
                kt = io.tile([D, KL], f32r, tag="kt")
                qt = io.tile([D, QL], f32r, tag="qt")
                if first:
                    # first exp needs qt + kt chunk 0 + mask. Issue them on
                    # three different sequencers so nothing serializes: qt on
                    # the ACT HWDGE ring (ACT seq is idle at t~0.3us; SP's
                    # preamble runs to ~1us), kt0 on SP, mask on GPSIMD SWDGE
                    nc.scalar.dma_start(out=qt, in_=qd[h])
                    nc.sync.dma_start(out=kt[:, 0:P], in_=kd[h][:, 0:P])
                    emit_mask()
                    if nck > 1:
                        nc.sync.dma_start(
                            out=kt[:, P : nck * P], in_=kd[h][:, P : nck * P]
                        )
                else:
                    nc.sync.dma_start(
                        out=kt[:, 0 : nck * P], in_=kd[h][:, 0 : nck * P]
                    )
                    nc.sync.dma_start(out=qt, in_=qd[h])
                v1_sb = io.tile([P, NCHUNK, D + 1], f32r, tag="v")
                nc.sync.dma_start(
                    out=v1_sb[:, 0:nck, :],
                    in_=vd[h].rearrange("c p m -> p c m")[:, 0:nck, :],
                )
                return qt, kt, v1_sb

            def emit_chunks(h, state, last=False):
                b = h // 2
                nck = nc_chunks[b]
                qt, kt, v1_sb = state
                # ---- per k-chunk: scores -> exp ----
                # (for the final head, P^T is split into per-q-half tiles so
                # the PV tail can start as soon as its half is done)
                if last:
                    pt0 = ptp.tile([P, NCHUNK, 512], f32r, tag="pt0")
                    pt1 = ptp.tile([P, NCHUNK, 512], f32r, tag="pt1")
                    pts = [pt0, pt1]
                else:
                    pt = ptp.tile([P, NCHUNK, QL], f32r, tag="pt")
                    pts = [pt[:, :, 0:512], pt[:, :, 512:QL]]
                for c in range(nck):
                    ps = ps_s.tile([P, QL], f32, tag="s")
                    nc.tensor.matmul(
                        ps[:, 0:512],
                        kt[:, c * P : (c + 1) * P],
                        qt[:, 0:512],
                        start=True, stop=True,
                    )
                    nc.tensor.matmul(
                        ps[:, 512:1024],
                        kt[:, c * P : (c + 1) * P],
                        qt[:, 512:1024],
                        start=True, stop=True,
                    )
                    if last:
                        for qh in range(2):
                            nc.scalar.activation(
                                pts[qh][:, c, :],
                                ps[:, qh * 512 : (qh + 1) * 512],
                                Exp, bias=mask_sb[:, b, c : c + 1],
                                scale=0.125,
                            )
                    else:
                        nc.scalar.activation(
                            pt[:, c, :], ps,
                            Exp, bias=mask_sb[:, b, c : c + 1], scale=0.125,
                        )
                return pts, v1_sb

            def emit_pvfin(h, pt_v, last=False):
                b = h // 2
                nck = nc_chunks[b]
                pts, v1_sb = pt_v
                # ---- PV (+sumexp via ones row) ----
                ots = []
                for qh in range(2):
                    po = ps_o.tile([D + 1, 512], f32, tag="o")
                    for c in range(nck):
                        nc.tensor.matmul(
                            po,
                            v1_sb[:, c, :],
                            pts[qh][:, c, :],
                            start=(c == 0), stop=(c == nck - 1),
                        )
                    oth = otp.tile([D + 1, 512], f32, tag="ot")
                    nc.vector.tensor_copy(oth, po)
                    ots.append(oth)
                # ---- transpose back, normalize, store ----
                fin = finp.tile([P, 4, 2, D], f32, tag="fin")
                for qg in range(2):
                    pf = ps_f.tile([P, 4, D + 1], f32, tag="pf")
                    for j in range(4):
                        nc.tensor.transpose(
                            pf[:, j, :], ots[qg][:, j * P : (j + 1) * P],
                            identity[0 : D + 1, 0 : D + 1],
                        )
                    rc = rcp.tile([P, 4], f32, tag="rc")
                    nc.vector.reciprocal(rc, pf[:, :, D : D + 1])
                    nc.vector.tensor_mul(
                        fin[:, 2 * qg : 2 * qg + 2, :, :],
                        pf[:, :, 0:D],
                        rc[:, :, None].broadcast_to([P, 4, D]),
                    )
                    if last:
                        # tail head: store each q-half as soon as it's ready
                        nc.sync.dma_start(
                            out=od[h].rearrange(
                                "(a p j) d -> p a j d", p=P, j=2
                            )[:, 2 * qg : 2 * qg + 2],
                            in_=fin[:, 2 * qg : 2 * qg + 2],
                        )
                if not last:
                    nc.sync.dma_start(
                        out=od[h].rearrange("(a p j) d -> p a j d", p=P, j=2),
                        in_=fin,
                    )

            # Interleave big and small heads: a head's finalize (DVE-heavy,
            # ~3us) hides under the NEXT head's exp phase only if that head
            # has enough chunks, so follow every small head with a big one
            # and end with the smallest (shortest un-hidden tail).
            by_size = sorted(range(HPC), key=lambda h: -nc_chunks[h // 2])
            big, small = by_size[: HPC // 2], by_size[HPC // 2 :]
            order = [h for pair in zip(big, small) for h in pair]
            pending_chunks = None
            pending_pv = None
            for h_rep in range(HPC * reps):
                h = order[h_rep % HPC]
                st = emit_front(h, first=(h_rep == 0))
                if pending_chunks is not None:
                    ch, cst = pending_chunks
                    if pending_pv is not None:
                        pv_h, pv_state = pending_pv
                        emit_chunks_out = emit_chunks(ch, cst)
                        emit_pvfin(pv_h, pv_state)
                    else:
                        emit_chunks_out = emit_chunks(ch, cst)
                    pending_pv = (ch, emit_chunks_out)
                pending_chunks = (h, st)
            ch, cst = pending_chunks
            out_last = emit_chunks(ch, cst, last=True)
            if pending_pv is not None:
                emit_pvfin(*pending_pv)
            emit_pvfin(ch, out_last, last=True)
    _split_excess_waits(nc)
    return nc


_CACHE = {}


def _get_nc(key, nc_chunks):
    if key not in _CACHE:
        _CACHE[key] = _build(nc_chunks)
    return _CACHE[key]


def _core_head_idx(c):
    return [b * NH + 2 * c + j for b in range(NB) for j in range(2)]


def _run(in_maps, nc, trace=False):
    from concourse.bass_utils import run_bass_kernel_spmd

    return run_bass_kernel_spmd(
        nc, in_maps, core_ids=list(range(NCORES)), trace=trace
    )


def _prepare(queries, keys, values, valid_lens):
    queries = np.asarray(queries, np.float32)
    keys = np.asarray(keys, np.float32)
    values = np.asarray(values, np.float32)
    vl = np.asarray(valid_lens).astype(np.int64)
    mask = np.where(
        np.arange(KL)[None, :] >= vl[:, None], np.float32(NEG), np.float32(0.0)
    ).astype(np.float32)
    # device layout [p, b*NCHUNK + c] = mask[b, c*128 + p]
    mask_dev = np.ascontiguousarray(
        mask.reshape(NB, NCHUNK, P).transpose(2, 0, 1).reshape(P, NB * NCHUNK)
    )
    nc_chunks = [max(1, int(min(NCHUNK, (int(v) + P - 1) // P))) for v in vl]
    bh = queries.shape[0]
    # Q^T / K^T with paired column order (see module docstring)
    qtp = np.ascontiguousarray(
        queries.reshape(bh, 4, P, 2, D).transpose(0, 4, 1, 3, 2).reshape(
            bh, D, QL
        )
    )
    ktp = np.ascontiguousarray(keys.transpose(0, 2, 1))
    # V with ones column: [BH, NCHUNK, P, D+1]
    v1 = np.concatenate(
        [values, np.ones((bh, KL, 1), np.float32)], axis=-1
    )
    v1p = np.ascontiguousarray(v1.reshape(bh, NCHUNK, P, D + 1))
    in_maps = []
    for c in range(NCORES):
        idx = _core_head_idx(c)
        in_maps.append(
            {
                "qt": qtp[idx],
                "kt": ktp[idx],
                "v": v1p[idx],
                "mask": mask_dev,
            }
        )
    return in_maps, nc_chunks, vl


def _gather(results, values, vl):
    out = np.empty((NB * NH, QL, D), np.float32)
    for c in range(NCORES):
        out[_core_head_idx(c)] = results[c]["out"]
    # fully-masked batches: reference softmax(-1e6 * ones) is uniform
    for b in range(NB):
        if vl[b] == 0:
            for hh in range(NH):
                bh = b * NH + hh
                out[bh] = np.asarray(values[bh], np.float32).mean(
                    axis=0, keepdims=True
                )
    return out


def kernel(queries, keys, values, valid_lens):
    in_maps, nc_chunks, vl = _prepare(queries, keys, values, valid_lens)
    nc = _get_nc(tuple(nc_chunks), nc_chunks)
    res = _run(in_maps, nc)
    return _gather(res.results, values, vl)


# revision 46
# speedup vs baseline: 1.0011x; 1.0011x over previous
"""Masked dot-product attention on 8 Trainium2 NeuronCores.

Problem shapes (hardcoded): queries/keys/values [128, 1024, 64] f32,
valid_lens [8] int (per-batch key valid length; BH = 8 batches x 16 heads).

Sharding: core c handles heads {b*16 + 2c, b*16 + 2c + 1} for all batches b
(16 heads/core, every batch present on every core -> uniform work, and one
compiled program serves all cores even with valid-len-dependent trip counts).

Host-side input prep (layout only; all attention math runs on device):
  - Q^T [BH, 64, 1024] with columns in "paired" order: column c*128+p holds
    query position (c//2)*256 + 2p + (c%2), so the output DMA writes >=512B
    contiguous runs (2x DMA bandwidth); the permutation is undone by the
    output access pattern. K^T [BH, 64, 1024] in natural column order
    (contiguous rows already give full DMA bandwidth, and natural order
    keeps valid-len truncation at 128-chunk granularity).
  - V is augmented with a ones column (softmax-denominator trick):
    [BH, 8, 128, 65], chunk-major.
  - mask is an additive bias laid out exactly as the device consumes it:
    [128, b*8+c] = 0 or -1e6 for key position c*128+p of batch b.

Per-head device pipeline (scores kept transposed, S^T[k, q]):
  per k-chunk c (only chunks below the batch's valid_len are computed):
    S^T[c] [128, 1024] = K^T_c.T @ Q^T            (PSUM, 2 matmuls, fp32r)
    P^T[c] = exp(S^T[c] * 1/8 + maskbias_c)       (ScalarE, bias = mask col)
  PV with ones-augmented V: out^T [65, q] += [V|1]_c.T @ P^T[c]; row 64
  accumulates sum(exp) = softmax denominator.
  PE-transpose out^T back to [q, 65]; reciprocal + scale on DVE -> [q, 64];
  DMA out (descriptors un-permute q).

fp32r (TF32-like, 4-byte) matmul inputs: 4x faster than fp32 on the PE,
HW-measured end-to-end rel err ~2e-4. No max-subtraction needed: scores are
O(10) so exp never overflows, masked entries give exactly 0. Fully-masked
batches (valid_len == 0) are patched on host to the reference's
uniform-softmax value.
"""

import numpy as np

P = 128          # partitions / k-chunk size
D = 64           # head dim
QL = 1024        # query length
KL = 1024        # key length
NB = 8           # batches
NH = 16          # heads per batch
NCORES = 8
HPC = 16         # heads per core
NCHUNK = KL // P # 8 k-chunks
NEG = -1.0e6

_POOLCFG = dict(io=3, pt=2, ot=4, fin=4, s=2, o=2, f=2)


def _split_excess_waits(nc, max_waits=1):
    """This walrus (gen3) accepts only one sync-wait per instruction, but Tile
    emits up to 2 on compute ops and 5+ on the kernel-tail drain. Hoist excess
    on_wait entries onto fresh InstEventSemaphore ops on the same engine,
    inserted immediately before the offending instruction (same semantics:
    the engine stalls on each wait sequentially)."""
    import bass_rust
    import concourse.mybir as mybir

    n_split = 0
    for func in nc.m.functions:
        for block in func.blocks:
            out = []
            changed = False
            for inst in block.instructions:
                si = getattr(inst, "sync_info", None)
                waits = list(si.on_wait) if si is not None else []
                if len(waits) > max_waits:
                    changed = True
                    for w in waits[:-max_waits]:
                        n_split += 1
                        out.append(
                            mybir.InstEventSemaphore(
                                name=f"waitsplit_{n_split}_{inst.name}",
                                engine=inst.engine,
                                ins=[],
                                outs=[],
                                sync_info=bass_rust.SyncInfo(
                                    on_wait=[w], on_update=[]
                                ),
                            )
                        )
                    inst.sync_info = bass_rust.SyncInfo(
                        on_wait=waits[-max_waits:], on_update=list(si.on_update)
                    )
                out.append(inst)
            if changed:
                block.instructions = out
    return n_split


def _build(nc_chunks=None, reps=1):
    """Build the Bass program. nc_chunks: per-batch count of 128-wide k-chunks
    to compute (valid-len truncation). reps>1 repeats the whole pipeline
    in-NEFF (only used for wall-clock delta timing experiments)."""
    import concourse.bass as bass
    import concourse.mybir as mybir
    from concourse.tile import TileContext
    from concourse.masks import make_identity

    if nc_chunks is None:
        nc_chunks = [NCHUNK] * NB

    f32 = mybir.dt.float32
    f32r = mybir.dt.float32r
    Exp = mybir.ActivationFunctionType.Exp

    nc = bass.Bass(trn_type="TRN2")
    qd = nc.dram_tensor("qt", [HPC, D, QL], f32r, kind="ExternalInput")
    kd = nc.dram_tensor("kt", [HPC, D, KL], f32r, kind="ExternalInput")
    vd = nc.dram_tensor("v", [HPC, NCHUNK, P, D + 1], f32r, kind="ExternalInput")
    md = nc.dram_tensor("mask", [P, NB * NCHUNK], f32, kind="ExternalInput")
    od = nc.dram_tensor("out", [HPC, QL, D], f32, kind="ExternalOutput")

    cfg = dict(_POOLCFG)
    with TileContext(nc) as tc:
        with (
            tc.tile_pool(name="consts", bufs=1) as consts,
            tc.tile_pool(name="io", bufs=cfg["io"]) as io,
            tc.tile_pool(name="pt", bufs=cfg["pt"]) as ptp,
            tc.tile_pool(name="ot", bufs=cfg["ot"]) as otp,
            tc.tile_pool(name="fin", bufs=cfg["fin"]) as finp,
            tc.tile_pool(name="rc", bufs=4) as rcp,
            tc.tile_pool(name="ps_s", bufs=cfg["s"], space="PSUM") as ps_s,
            tc.tile_pool(name="ps_o", bufs=cfg["o"], space="PSUM") as ps_o,
            tc.tile_pool(name="ps_f", bufs=cfg["f"], space="PSUM") as ps_f,
        ):
            identity = consts.tile([P, P], f32)
            make_identity(nc, identity)
            # prime the ScalarE exp table load (~2.7us) so it overlaps the
            # first input DMAs instead of stalling the first real exp
            scratch = consts.tile([1, 1], f32)
            nc.vector.memset(scratch, 0.0)
            nc.scalar.activation(scratch, scratch, Exp)
            # prime the PE clock ramp with short dummy matmuls sized to end
            # right as the first real operands land (PE is FIFO: too many
            # dummies would delay the real matmuls)
            warm = ps_f.tile([1, D], f32, tag="pf")
            for _ in range(7):
                nc.tensor.matmul(
                    warm, identity[:, 0:1], identity[:, 0:D],
                    start=True, stop=True,
                )
            mask_sb = consts.tile([P, NB, NCHUNK], f32)

            def emit_mask():
                # SWDGE via the idle GPSIMD engine: issues in parallel with
                # the SP-sequencer DMAs feeding the first matmuls
                nc.gpsimd.dma_start(
                    out=mask_sb, in_=md.rearrange("p (b c) -> p b c", b=NB)
                )

            def emit_front(h, first=False):
                b = h // 2
                nck = nc_chunks[b]
                kt = io.tile([D, KL], f32r, tag="kt")
                qt = io.tile([D, QL], f32r, tag="qt")
                if first:
                    # first exp needs qt + kt chunk 0 + mask. Issue them on
                    # three different sequencers so nothing serializes: qt on
                    # the ACT HWDGE ring (ACT seq is idle at t~0.3us; SP's
                    # preamble runs to ~1us), kt0 on SP, mask on GPSIMD SWDGE
                    nc.scalar.dma_start(out=qt, in_=qd[h])
                    nc.sync.dma_start(out=kt[:, 0:P], in_=kd[h][:, 0:P])
                    emit_mask()
                    if nck > 1:
                        nc.sync.dma_start(
                            out=kt[:, P : nck * P], in_=kd[h][:, P : nck * P]
                        )
                else:
                    nc.sync.dma_start(
                        out=kt[:, 0 : nck * P], in_=kd[h][:, 0 : nck * P]
                    )
                    nc.sync.dma_start(out=qt, in_=qd[h])
                v1_sb = io.tile([P, NCHUNK, D + 1], f32r, tag="v")
                nc.sync.dma_start(
                    out=v1_sb[:, 0:nck, :],
                    in_=vd[h].rearrange("c p m -> p c m")[:, 0:nck, :],
                )
                return qt, kt, v1_sb

            def emit_chunks(h, state, last=False):
                b = h // 2
                nck = nc_chunks[b]
                qt, kt, v1_sb = state
                # ---- per k-chunk: scores -> exp ----
                # (for the final head, P^T is split into per-q-half tiles so
                # the PV tail can start as soon as its half is done)
                if last:
                    pt0 = ptp.tile([P, NCHUNK, 512], f32r, tag="pt0")
                    pt1 = ptp.tile([P, NCHUNK, 512], f32r, tag="pt1")
                    pts = [pt0, pt1]
                else:
                    pt = ptp.tile([P, NCHUNK, QL], f32r, tag="pt")
                    pts = [pt[:, :, 0:512], pt[:, :, 512:QL]]
                for c in range(nck):
                    ps = ps_s.tile([P, QL], f32, tag="s")
                    nc.tensor.matmul(
                        ps[:, 0:512],
                        kt[:, c * P : (c + 1) * P],
                        qt[:, 0:512],
                        start=True, stop=True,
                    )
                    nc.tensor.matmul(
                        ps[:, 512:1024],
                        kt[:, c * P : (c + 1) * P],
                        qt[:, 512:1024],
                        start=True, stop=True,
                    )
                    if last:
                        for qh in range(2):
                            nc.scalar.activation(
                                pts[qh][:, c, :],
                                ps[:, qh * 512 : (qh + 1) * 512],
                                Exp, bias=mask_sb[:, b, c : c + 1],
                                scale=0.125,
                            )
                    else:
                        nc.scalar.activation(
                            pt[:, c, :], ps,
                            Exp, bias=mask_sb[:, b, c : c + 1], scale=0.125,
                        )
                return pts, v1_sb

            def emit_pvfin(h, pt_v, last=False):
                b = h // 2
                nck = nc_chunks[b]
                pts, v1_sb = pt_v
                # ---- PV (+sumexp via ones row) ----
                ots = []
                for qh in range(2):
                    po = ps_o.tile([D + 1, 512], f32, tag="o")
                    for c in range(nck):
                        nc.tensor.matmul(
                            po,
                            v1_sb[:, c, :],
                            pts[qh][:, c, :],
                            start=(c == 0), stop=(c == nck - 1),
                        )
                    oth = otp.tile([D + 1, 512], f32, tag="ot")
                    nc.vector.tensor_copy(oth, po)
                    ots.append(oth)
                # ---- transpose back, normalize, store ----
                fin = finp.tile([P, 4, 2, D], f32, tag="fin")
                for qg in range(2):
                    pf = ps_f.tile([P, 4, D + 1], f32, tag="pf")
                    for j in range(4):
                        nc.tensor.transpose(
                            pf[:, j, :], ots[qg][:, j * P : (j + 1) * P],
                            identity[0 : D + 1, 0 : D + 1],
                        )
                    rc = rcp.tile([P, 4], f32, tag="rc")
                    nc.vector.reciprocal(rc, pf[:, :, D : D + 1])
                    nc.vector.tensor_mul(
                        fin[:, 2 * qg : 2 * qg + 2, :, :],
                        pf[:, :, 0:D],
                        rc[:, :, None].broadcast_to([P, 4, D]),
                    )
                    if last:
                        # tail head: store each q-half as soon as it's ready
                        nc.sync.dma_start(
                            out=od[h].rearrange(
                                "(a p j) d -> p a j d", p=P, j=2
                            )[:, 2 * qg : 2 * qg + 2],
                            in_=fin[:, 2 * qg : 2 * qg + 2],
                        )
                if not last:
                    nc.sync.dma_start(
                        out=od[h].rearrange("(a p j) d -> p a j d", p=P, j=2),
                        in_=fin,
                    )

            # Interleave big and small heads: a head's finalize (DVE-heavy,
            # ~3us) hides under the NEXT head's exp phase only if that head
            # has enough chunks, so follow every small head with a big one
            # and end with the smallest (shortest un-hidden tail).
            by_size = sorted(range(HPC), key=lambda h: -nc_chunks[h // 2])
            big, small = by_size[: HPC // 2], by_size[HPC // 2 :]
            order = [h for pair in zip(big, small) for h in pair]
            pending_chunks = None
            pending_pv = None
            for h_rep in range(HPC * reps):
                h = order[h_rep % HPC]
                st = emit_front(h, first=(h_rep == 0))
                if pending_chunks is not None:
                    ch, cst = pending_chunks
                    if pending_pv is not None:
                        pv_h, pv_state = pending_pv
                        emit_chunks_out = emit_chunks(ch, cst)
                        emit_pvfin(pv_h, pv_state)
                    else:
                        emit_chunks_out = emit_chunks(ch, cst)
                    pending_pv = (ch, emit_chunks_out)
                pending_chunks = (h, st)
            ch, cst = pending_chunks
            out_last = emit_chunks(ch, cst, last=True)
            if pending_pv is not None:
                emit_pvfin(*pending_pv)
            emit_pvfin(ch, out_last, last=True)
    _split_excess_waits(nc)
    return nc


_CACHE = {}


def _get_nc(key, nc_chunks):
    if key not in _CACHE:
        _CACHE[key] = _build(nc_chunks)
    return _CACHE[key]


def _core_head_idx(c):
    return [b * NH + 2 * c + j for b in range(NB) for j in range(2)]


def _run(in_maps, nc, trace=False):
    from concourse.bass_utils import run_bass_kernel_spmd

    return run_bass_kernel_spmd(
        nc, in_maps, core_ids=list(range(NCORES)), trace=trace
    )


def _prepare(queries, keys, values, valid_lens):
    queries = np.asarray(queries, np.float32)
    keys = np.asarray(keys, np.float32)
    values = np.asarray(values, np.float32)
    vl = np.asarray(valid_lens).astype(np.int64)
    mask = np.where(
        np.arange(KL)[None, :] >= vl[:, None], np.float32(NEG), np.float32(0.0)
    ).astype(np.float32)
    # device layout [p, b*NCHUNK + c] = mask[b, c*128 + p]
    mask_dev = np.ascontiguousarray(
        mask.reshape(NB, NCHUNK, P).transpose(2, 0, 1).reshape(P, NB * NCHUNK)
    )
    nc_chunks = [max(1, int(min(NCHUNK, (int(v) + P - 1) // P))) for v in vl]
    bh = queries.shape[0]
    # Q^T / K^T with paired column order (see module docstring)
    qtp = np.ascontiguousarray(
        queries.reshape(bh, 4, P, 2, D).transpose(0, 4, 1, 3, 2).reshape(
            bh, D, QL
        )
    )
    ktp = np.ascontiguousarray(keys.transpose(0, 2, 1))
    # V with ones column: [BH, NCHUNK, P, D+1]
    v1 = np.concatenate(
        [values, np.ones((bh, KL, 1), np.float32)], axis=-1
    )
    v1p = np.ascontiguousarray(v1.reshape(bh, NCHUNK, P, D + 1))
    in_maps = []
    for c in range(NCORES):
        idx = _core_head_idx(c)
        in_maps.append(
            {
                "qt": qtp[idx],
                "kt": ktp[idx],
                "v": v1p[idx],
                "mask": mask_dev,
            }
        )
    return in_maps, nc_chunks, vl


def _gather(results, values, vl):
    out = np.empty((NB * NH, QL, D), np.float32)
    for c in range(NCORES):
        out[_core_head_idx(c)] = results[c]["out"]
    # fully-masked batches: reference softmax(-1e6 * ones) is uniform
    for b in range(NB):
        if vl[b] == 0:
            for hh in range(NH):
                bh = b * NH + hh
                out[bh] = np.asarray(values[bh], np.float32).mean(
                    axis=0, keepdims=True
                )
    return out


def kernel(queries, keys, values, valid_lens):
    in_maps, nc_chunks, vl = _prepare(queries, keys, values, valid_lens)
    nc = _get_nc(tuple(nc_chunks), nc_chunks)
    res = _run(in_maps, nc)
    return _gather(res.results, values, vl)


# revision 47
# speedup vs baseline: 1.0014x; 1.0003x over previous
"""Masked dot-product attention on 8 Trainium2 NeuronCores.

Problem shapes (hardcoded): queries/keys/values [128, 1024, 64] f32,
valid_lens [8] int (per-batch key valid length; BH = 8 batches x 16 heads).

Sharding: core c handles heads {b*16 + 2c, b*16 + 2c + 1} for all batches b
(16 heads/core, every batch present on every core -> uniform work, and one
compiled program serves all cores even with valid-len-dependent trip counts).

Host-side input prep (layout only; all attention math runs on device):
  - Q^T [BH, 64, 1024] with columns in "paired" order: column c*128+p holds
    query position (c//2)*256 + 2p + (c%2), so the output DMA writes >=512B
    contiguous runs (2x DMA bandwidth); the permutation is undone by the
    output access pattern. K^T [BH, 64, 1024] in natural column order
    (contiguous rows already give full DMA bandwidth, and natural order
    keeps valid-len truncation at 128-chunk granularity).
  - V is augmented with a ones column (softmax-denominator trick):
    [BH, 8, 128, 65], chunk-major.
  - mask is an additive bias laid out exactly as the device consumes it:
    [128, b*8+c] = 0 or -1e6 for key position c*128+p of batch b.

Per-head device pipeline (scores kept transposed, S^T[k, q]):
  per k-chunk c (only chunks below the batch's valid_len are computed):
    S^T[c] [128, 1024] = K^T_c.T @ Q^T            (PSUM, 2 matmuls, fp32r)
    P^T[c] = exp(S^T[c] * 1/8 + maskbias_c)       (ScalarE, bias = mask col)
  PV with ones-augmented V: out^T [65, q] += [V|1]_c.T @ P^T[c]; row 64
  accumulates sum(exp) = softmax denominator.
  PE-transpose out^T back to [q, 65]; reciprocal + scale on DVE -> [q, 64];
  DMA out (descriptors un-permute q).

fp32r (TF32-like, 4-byte) matmul inputs: 4x faster than fp32 on the PE,
HW-measured end-to-end rel err ~2e-4. No max-subtraction needed: scores are
O(10) so exp never overflows, masked entries give exactly 0. Fully-masked
batches (valid_len == 0) are patched on host to the reference's
uniform-softmax value.
"""

import numpy as np

P = 128          # partitions / k-chunk size
D = 64           # head dim
QL = 1024        # query length
KL = 1024        # key length
NB = 8           # batches
NH = 16          # heads per batch
NCORES = 8
HPC = 16         # heads per core
NCHUNK = KL // P # 8 k-chunks
NEG = -1.0e6

_POOLCFG = dict(io=3, pt=2, ot=4, fin=4, s=2, o=2, f=2)


def _split_excess_waits(nc, max_waits=1):
    """This walrus (gen3) accepts only one sync-wait per instruction, but Tile
    emits up to 2 on compute ops and 5+ on the kernel-tail drain. Hoist excess
    on_wait entries onto fresh InstEventSemaphore ops on the same engine,
    inserted immediately before the offending instruction (same semantics:
    the engine stalls on each wait sequentially)."""
    import bass_rust
    import concourse.mybir as mybir

    n_split = 0
    for func in nc.m.functions:
        for block in func.blocks:
            out = []
            changed = False
            for inst in block.instructions:
                si = getattr(inst, "sync_info", None)
                waits = list(si.on_wait) if si is not None else []
                if len(waits) > max_waits:
                    changed = True
                    for w in waits[:-max_waits]:
                        n_split += 1
                        out.append(
                            mybir.InstEventSemaphore(
                                name=f"waitsplit_{n_split}_{inst.name}",
                                engine=inst.engine,
                                ins=[],
                                outs=[],
                                sync_info=bass_rust.SyncInfo(
                                    on_wait=[w], on_update=[]
                                ),
                            )
                        )
                    inst.sync_info = bass_rust.SyncInfo(
                        on_wait=waits[-max_waits:], on_update=list(si.on_update)
                    )
                out.append(inst)
            if changed:
                block.instructions = out
    return n_split


def _build(nc_chunks=None, reps=1):
    """Build the Bass program. nc_chunks: per-batch count of 128-wide k-chunks
    to compute (valid-len truncation). reps>1 repeats the whole pipeline
    in-NEFF (only used for wall-clock delta timing experiments)."""
    import concourse.bass as bass
    import concourse.mybir as mybir
    from concourse.tile import TileContext
    from concourse.masks import make_identity

    if nc_chunks is None:
        nc_chunks = [NCHUNK] * NB

    f32 = mybir.dt.float32
    f32r = mybir.dt.float32r
    Exp = mybir.ActivationFunctionType.Exp

    nc = bass.Bass(trn_type="TRN2")
    qd = nc.dram_tensor("qt", [HPC, D, QL], f32r, kind="ExternalInput")
    kd = nc.dram_tensor("kt", [HPC, D, KL], f32r, kind="ExternalInput")
    vd = nc.dram_tensor("v", [HPC, NCHUNK, P, D + 1], f32r, kind="ExternalInput")
    md = nc.dram_tensor("mask", [P, NB * NCHUNK], f32, kind="ExternalInput")
    od = nc.dram_tensor("out", [HPC, QL, D], f32, kind="ExternalOutput")

    cfg = dict(_POOLCFG)
    with TileContext(nc) as tc:
        with (
            tc.tile_pool(name="consts", bufs=1) as consts,
            tc.tile_pool(name="io", bufs=cfg["io"]) as io,
            tc.tile_pool(name="pt", bufs=cfg["pt"]) as ptp,
            tc.tile_pool(name="ot", bufs=cfg["ot"]) as otp,
            tc.tile_pool(name="fin", bufs=cfg["fin"]) as finp,
            tc.tile_pool(name="rc", bufs=4) as rcp,
            tc.tile_pool(name="ps_s", bufs=cfg["s"], space="PSUM") as ps_s,
            tc.tile_pool(name="ps_o", bufs=cfg["o"], space="PSUM") as ps_o,
            tc.tile_pool(name="ps_f", bufs=cfg["f"], space="PSUM") as ps_f,
        ):
            # the mask load goes FIRST on the GPSIMD queue (SWDGE issues in
            # parallel with the SP/ACT-sequencer DMAs feeding the first
            # matmuls); the identity build follows — it is only needed by the
            # first head's final transposes, much later
            mask_sb = consts.tile([P, NB, NCHUNK], f32)
            nc.gpsimd.dma_start(
                out=mask_sb, in_=md.rearrange("p (b c) -> p b c", b=NB)
            )
            identity = consts.tile([P, P], f32)
            make_identity(nc, identity)
            # prime the ScalarE exp table load (~2.7us) so it overlaps the
            # first input DMAs instead of stalling the first real exp
            scratch = consts.tile([1, 1], f32)
            nc.vector.memset(scratch, 0.0)
            nc.scalar.activation(scratch, scratch, Exp)
            # prime the PE clock ramp with short dummy matmuls sized to end
            # right as the first real operands land (PE is FIFO: too many
            # dummies would delay the real matmuls)
            warm = ps_f.tile([1, D], f32, tag="pf")
            for _ in range(4):
                nc.tensor.matmul(
                    warm, identity[:, 0:1], identity[:, 0:D],
                    start=True, stop=True,
                )

            def emit_mask():
                pass

            def emit_front(h, first=False):
                b = h // 2
                nck = nc_chunks[b]
                kt = io.tile([D, KL], f32r, tag="kt")
                qt = io.tile([D, QL], f32r, tag="qt")
                if first:
                    # first exp needs qt + kt chunk 0 + mask. Issue them on
                    # three different sequencers so nothing serializes: qt on
                    # the ACT HWDGE ring (ACT seq is idle at t~0.3us; SP's
                    # preamble runs to ~1us), kt0 on SP, mask on GPSIMD SWDGE
                    nc.scalar.dma_start(out=qt, in_=qd[h])
                    nc.sync.dma_start(out=kt[:, 0:P], in_=kd[h][:, 0:P])
                    emit_mask()
                    if nck > 1:
                        nc.sync.dma_start(
                            out=kt[:, P : nck * P], in_=kd[h][:, P : nck * P]
                        )
                else:
                    nc.sync.dma_start(
                        out=kt[:, 0 : nck * P], in_=kd[h][:, 0 : nck * P]
                    )
                    nc.sync.dma_start(out=qt, in_=qd[h])
                v1_sb = io.tile([P, NCHUNK, D + 1], f32r, tag="v")
                nc.sync.dma_start(
                    out=v1_sb[:, 0:nck, :],
                    in_=vd[h].rearrange("c p m -> p c m")[:, 0:nck, :],
                )
                return qt, kt, v1_sb

            def emit_chunks(h, state, last=False):
                b = h // 2
                nck = nc_chunks[b]
                qt, kt, v1_sb = state
                # ---- per k-chunk: scores -> exp ----
                # (for the final head, P^T is split into per-q-half tiles so
                # the PV tail can start as soon as its half is done)
                if last:
                    pt0 = ptp.tile([P, NCHUNK, 512], f32r, tag="pt0")
                    pt1 = ptp.tile([P, NCHUNK, 512], f32r, tag="pt1")
                    pts = [pt0, pt1]
                else:
                    pt = ptp.tile([P, NCHUNK, QL], f32r, tag="pt")
                    pts = [pt[:, :, 0:512], pt[:, :, 512:QL]]
                for c in range(nck):
                    ps = ps_s.tile([P, QL], f32, tag="s")
                    nc.tensor.matmul(
                        ps[:, 0:512],
                        kt[:, c * P : (c + 1) * P],
                        qt[:, 0:512],
                        start=True, stop=True,
                    )
                    nc.tensor.matmul(
                        ps[:, 512:1024],
                        kt[:, c * P : (c + 1) * P],
                        qt[:, 512:1024],
                        start=True, stop=True,
                    )
                    if last:
                        for qh in range(2):
                            nc.scalar.activation(
                                pts[qh][:, c, :],
                                ps[:, qh * 512 : (qh + 1) * 512],
                                Exp, bias=mask_sb[:, b, c : c + 1],
                                scale=0.125,
                            )
                    else:
                        nc.scalar.activation(
                            pt[:, c, :], ps,
                            Exp, bias=mask_sb[:, b, c : c + 1], scale=0.125,
                        )
                return pts, v1_sb

            def emit_pvfin(h, pt_v, last=False):
                b = h // 2
                nck = nc_chunks[b]
                pts, v1_sb = pt_v
                # ---- PV (+sumexp via ones row) ----
                ots = []
                for qh in range(2):
                    po = ps_o.tile([D + 1, 512], f32, tag="o")
                    for c in range(nck):
                        nc.tensor.matmul(
                            po,
                            v1_sb[:, c, :],
                            pts[qh][:, c, :],
                            start=(c == 0), stop=(c == nck - 1),
                        )
                    oth = otp.tile([D + 1, 512], f32, tag="ot")
                    nc.vector.tensor_copy(oth, po)
                    ots.append(oth)
                # ---- transpose back, normalize, store ----
                fin = finp.tile([P, 4, 2, D], f32, tag="fin")
                for qg in range(2):
                    pf = ps_f.tile([P, 4, D + 1], f32, tag="pf")
                    for j in range(4):
                        nc.tensor.transpose(
                            pf[:, j, :], ots[qg][:, j * P : (j + 1) * P],
                            identity[0 : D + 1, 0 : D + 1],
                        )
                    rc = rcp.tile([P, 4], f32, tag="rc")
                    nc.vector.reciprocal(rc, pf[:, :, D : D + 1])
                    nc.vector.tensor_mul(
                        fin[:, 2 * qg : 2 * qg + 2, :, :],
                        pf[:, :, 0:D],
                        rc[:, :, None].broadcast_to([P, 4, D]),
                    )
                    if last:
                        # tail head: store each q-half as soon as it's ready
                        nc.sync.dma_start(
                            out=od[h].rearrange(
                                "(a p j) d -> p a j d", p=P, j=2
                            )[:, 2 * qg : 2 * qg + 2],
                            in_=fin[:, 2 * qg : 2 * qg + 2],
                        )
                if not last:
                    nc.sync.dma_start(
                        out=od[h].rearrange("(a p j) d -> p a j d", p=P, j=2),
                        in_=fin,
                    )

            # Interleave big and small heads: a head's finalize (DVE-heavy,
            # ~3us) hides under the NEXT head's exp phase only if that head
            # has enough chunks, so follow every small head with a big one
            # and end with the smallest (shortest un-hidden tail).
            by_size = sorted(range(HPC), key=lambda h: -nc_chunks[h // 2])
            big, small = by_size[: HPC // 2], by_size[HPC // 2 :]
            order = [h for pair in zip(big, small) for h in pair]
            pending_chunks = None
            pending_pv = None
            for h_rep in range(HPC * reps):
                h = order[h_rep % HPC]
                st = emit_front(h, first=(h_rep == 0))
                if pending_chunks is not None:
                    ch, cst = pending_chunks
                    if pending_pv is not None:
                        pv_h, pv_state = pending_pv
                        emit_chunks_out = emit_chunks(ch, cst)
                        emit_pvfin(pv_h, pv_state)
                    else:
                        emit_chunks_out = emit_chunks(ch, cst)
                    pending_pv = (ch, emit_chunks_out)
                pending_chunks = (h, st)
            ch, cst = pending_chunks
            out_last = emit_chunks(ch, cst, last=True)
            if pending_pv is not None:
                emit_pvfin(*pending_pv)
            emit_pvfin(ch, out_last, last=True)
    _split_excess_waits(nc)
    return nc


_CACHE = {}


def _get_nc(key, nc_chunks):
    if key not in _CACHE:
        _CACHE[key] = _build(nc_chunks)
    return _CACHE[key]


def _core_head_idx(c):
    return [b * NH + 2 * c + j for b in range(NB) for j in range(2)]


def _run(in_maps, nc, trace=False):
    from concourse.bass_utils import run_bass_kernel_spmd

    return run_bass_kernel_spmd(
        nc, in_maps, core_ids=list(range(NCORES)), trace=trace
    )


def _prepare(queries, keys, values, valid_lens):
    queries = np.asarray(queries, np.float32)
    keys = np.asarray(keys, np.float32)
    values = np.asarray(values, np.float32)
    vl = np.asarray(valid_lens).astype(np.int64)
    mask = np.where(
        np.arange(KL)[None, :] >= vl[:, None], np.float32(NEG), np.float32(0.0)
    ).astype(np.float32)
    # device layout [p, b*NCHUNK + c] = mask[b, c*128 + p]
    mask_dev = np.ascontiguousarray(
        mask.reshape(NB, NCHUNK, P).transpose(2, 0, 1).reshape(P, NB * NCHUNK)
    )
    nc_chunks = [max(1, int(min(NCHUNK, (int(v) + P - 1) // P))) for v in vl]
    bh = queries.shape[0]
    # Q^T / K^T with paired column order (see module docstring)
    qtp = np.ascontiguousarray(
        queries.reshape(bh, 4, P, 2, D).transpose(0, 4, 1, 3, 2).reshape(
            bh, D, QL
        )
    )
    ktp = np.ascontiguousarray(keys.transpose(0, 2, 1))
    # V with ones column: [BH, NCHUNK, P, D+1]
    v1 = np.concatenate(
        [values, np.ones((bh, KL, 1), np.float32)], axis=-1
    )
    v1p = np.ascontiguousarray(v1.reshape(bh, NCHUNK, P, D + 1))
    in_maps = []
    for c in range(NCORES):
        idx = _core_head_idx(c)
        in_maps.append(
            {
                "qt": qtp[idx],
                "kt": ktp[idx],
                "v": v1p[idx],
                "mask": mask_dev,
            }
        )
    return in_maps, nc_chunks, vl


def _gather(results, values, vl):
    out = np.empty((NB * NH, QL, D), np.float32)
    for c in range(NCORES):
        out[_core_head_idx(c)] = results[c]["out"]
    # fully-masked batches: reference softmax(-1e6 * ones) is uniform
    for b in range(NB):
        if vl[b] == 0:
            for hh in range(NH):
                bh = b * NH + hh
                out[bh] = np.asarray(values[bh], np.float32).mean(
                    axis=0, keepdims=True
                )
    return out


def kernel(queries, keys, values, valid_lens):
    in_maps, nc_chunks, vl = _prepare(queries, keys, values, valid_lens)
    nc = _get_nc(tuple(nc_chunks), nc_chunks)
    res = _run(in_maps, nc)
    return _gather(res.results, values, vl)
